# revision 19
# baseline (speedup 1.0000x reference)
"""Trainium2 Bass kernel for NonlinearElectronicEmbedding (segment softmax).

Design ("T2", transposed / padding-free):
  - 512 molecules -> 64 consecutive molecules per core (8 cores). Atoms of
    a core's molecules form one contiguous run (batch_seg sorted); x is
    shipped TRANSPOSED (features on partitions, atoms on the free axis) in
    fp16, so there is no 128-atom padding at all.
  - Prelude computes the k/v tables from E via the ResidualMLPs in
    transposed layout (features on partitions), fusing Wq and kbo@Wq into
    the k-table:  dot(a) = x(a) . (k_mol @ Wq)[seg(a)].
  - Main loop over "supers" of 1024 atoms:
      dots  = kqT^T @ xT           (PE, all 64 molecules at once, fp16)
      e     = exp(dots/16)         (ACT, PSUM->SBUF fp16)
      S     = e * mask, partial = rowsum(S)   (DVE stt fused accum)
      anorm += partial; r = 1/(anorm+eps)     (tiny DVE)
      S[s-1] *= r  (per-partition scalar; every molecule of super s-1 is
                    closed by the end of super s since molecules < 1024)
      outT[s-1] = v16^T @ S[s-1]   (PE outer product, K=64)
      copy PSUM->SBUF fp16 (split ACT/DVE), DMA out.
  - mask is a host-built fp16 0/1 band matrix [64, NCpad] (bs sorted ->
    band). Garbage dot rows (wrong molecules) are zeroed by it; softmax
    shift invariance makes the seg_max pass unnecessary (args bounded).
  - Host does only layout work: transpose+fp16 cast in, transpose+fp32
    cast out.
HBM traffic/core ~ 26+6+26 MB (x + mask + out, fp16) -> memory roofline.
"""

import numpy as np

F = 256
B = 512
NCORES = 8
BC = B // NCORES  # molecules per core
P = 128
SUP = 1024        # atoms per super-group (2 PSUM banks of dots)
HB = SUP // 2     # 512, one PSUM bank
BETA = 1.702
EPS = 1e-8
INV_SQRT_F = 1.0 / 16.0


def _build_program(nsup):
    import concourse.bacc as bacc
    import concourse.mybir as mybir
    import concourse.tile as tile

    dt = mybir.dt
    f32 = dt.float32
    f16 = dt.bfloat16
    AF = mybir.ActivationFunctionType
    ALU = mybir.AluOpType

    NCpad = nsup * SUP

    nc = bacc.Bacc(trn_type="TRN2")

    x_h = nc.dram_tensor("x", [P, nsup * 2 * SUP], f16, kind="ExternalInput")
    mk_h = nc.dram_tensor("mk", [1, NCpad], f16, kind="ExternalInput")
    iota_h = nc.dram_tensor("iota", [BC, 1], f32, kind="ExternalInput")
    ev_h = nc.dram_tensor("ev", [1, BC], f32, kind="ExternalInput")
    wkf_h = nc.dram_tensor("wkf", [1, F], f32, kind="ExternalInput")
    wvf_h = nc.dram_tensor("wvf", [1, F], f32, kind="ExternalInput")
    kw1_h = nc.dram_tensor("kw1", [P, 2, 2, P], f32, kind="ExternalInput")
    kw2_h = nc.dram_tensor("kw2", [P, 2, 2, P], f32, kind="ExternalInput")
    vw1_h = nc.dram_tensor("vw1", [P, 2, 2, P], f32, kind="ExternalInput")
    vw2_h = nc.dram_tensor("vw2", [P, 2, 2, P], f32, kind="ExternalInput")
    woqk_h = nc.dram_tensor("woqk", [P, 2, 2, P], f32, kind="ExternalInput")
    wovv_h = nc.dram_tensor("wovv", [P, 2, 2, P], f32, kind="ExternalInput")
    bq_h = nc.dram_tensor("bq", [1, F], f32, kind="ExternalInput")
    # biases: [P, 2] chunked; *_s pre-multiplied by BETA, *_u raw
    bkfs_h = nc.dram_tensor("bkfs", [P, 2], f32, kind="ExternalInput")
    bkfu_h = nc.dram_tensor("bkfu", [P, 2], f32, kind="ExternalInput")
    kb1s_h = nc.dram_tensor("kb1s", [P, 2], f32, kind="ExternalInput")
    kb1u_h = nc.dram_tensor("kb1u", [P, 2], f32, kind="ExternalInput")
    kb2u_h = nc.dram_tensor("kb2u", [P, 2], f32, kind="ExternalInput")
    out_h = nc.dram_tensor("out", [P, nsup * 2 * SUP], f16,
                           kind="ExternalOutput")

    # per-super interleaved layout: row p holds [s][c][j] so one DMA moves
    # 4KB contiguous per partition per super
    xv = x_h[:].rearrange("p (s c j) -> p s c j", s=nsup, c=2)
    ov = out_h[:].rearrange("p (s c j) -> p s c j", s=nsup, c=2)

    with tile.TileContext(nc) as tc:
        with (
            tc.tile_pool(name="singles", bufs=1) as sg,
            tc.tile_pool(name="xpool", bufs=4) as xp,
            tc.tile_pool(name="mpool", bufs=4) as mp,
            tc.tile_pool(name="epool", bufs=2) as ep,
            tc.tile_pool(name="spool", bufs=5) as sp_,
            tc.tile_pool(name="opool", bufs=4) as op,
            tc.tile_pool(name="rpool", bufs=5) as rp,
        ):
            def load(name, h, shape):
                t_ = sg.tile(shape, f32, tag=name, name=name)
                nc.sync.dma_start(out=t_[:], in_=h[:])
                return t_

            ev_sb = load("ev", ev_h, [1, BC])
            iota_sb = load("iota", iota_h, [BC, 1])
            wkf_sb = load("wkf", wkf_h, [1, F])
            wvf_sb = load("wvf", wvf_h, [1, F])
            kw1_sb = load("kw1", kw1_h, [P, 2, 2, P])
            kw2_sb = load("kw2", kw2_h, [P, 2, 2, P])
            vw1_sb = load("vw1", vw1_h, [P, 2, 2, P])
            vw2_sb = load("vw2", vw2_h, [P, 2, 2, P])
            woqk_sb = load("woqk", woqk_h, [P, 2, 2, P])
            wovv_sb = load("wovv", wovv_h, [P, 2, 2, P])
            bq_sb = load("bq", bq_h, [1, F])
            bkfs_sb = load("bkfs", bkfs_h, [P, 2])
            bkfu_sb = load("bkfu", bkfu_h, [P, 2])
            kb1s_sb = load("kb1s", kb1s_h, [P, 2])
            kb1u_sb = load("kb1u", kb1u_h, [P, 2])
            kb2u_sb = load("kb2u", kb2u_h, [P, 2])

            ones1 = sg.tile([1, BC], f32)
            nc.vector.memset(ones1[:], 1.0)

            kqT16 = sg.tile([P, 2, BC], f16)   # kqT16[f', c, b]
            v16 = sg.tile([BC, 2, P], f16)     # v16[b, c, f']
            anorm_run = sg.tile([BC, 1], f32)
            nc.vector.memset(anorm_run[:], 0.0)

            # ---- prelude: ResidualMLP in transposed layout ----
            # swish(y) = y * sigmoid(BETA*y); h_psum holds y - b.
            def swishT(c, h_psum, bs_ap, bu_ap, pre, keep_hb=False):
                sig = pre.tile([P, BC], f32, tag=f"sig_{c}", name="sig")
                nc.scalar.activation(sig[:], h_psum[:], AF.Sigmoid,
                                     bias=bs_ap if bs_ap is not None else 0.0,
                                     scale=BETA)
                if bu_ap is not None:
                    hb = pre.tile([P, BC], f32, tag=f"hb_{c}", name="hb")
                    nc.vector.tensor_scalar_add(hb[:], h_psum[:], bu_ap)
                elif keep_hb:
                    hb = pre.tile([P, BC], f32, tag=f"hb_{c}", name="hb")
                    nc.vector.tensor_copy(hb[:], h_psum[:])
                else:
                    hb = h_psum
                s = pre.tile([P, BC], f32, tag=f"s_{c}", name="s")
                nc.vector.tensor_mul(s[:], hb[:], sig[:])
                return (s, hb) if keep_hb else (s, None)

            def resmlp_T(wf_sb, b0s, b0u, w1_sb, b1s, b1u, w2_sb, b2u,
                         pre, ppre, branch):
                h0, s1, h1, s2, h2, s3, hb0 = [], [], [], [], [], [], []
                for c in (0, 1):
                    t_ = ppre.tile([P, BC], f32, tag=f"h0_{c}", name="h0")
                    nc.tensor.matmul(t_[:], wf_sb[0:1, c * P:(c + 1) * P],
                                     ev_sb[:], start=True, stop=True)
                    h0.append(t_)
                for c in (0, 1):
                    s, hb = swishT(
                        f"a{c}", h0[c],
                        b0s[:, c:c + 1] if b0s is not None else None,
                        b0u[:, c:c + 1] if b0u is not None else None,
                        pre, keep_hb=True)
                    s1.append(s)
                    hb0.append(hb if hb is not None else h0[c])
                for m in (0, 1):
                    t_ = ppre.tile([P, BC], f32, tag=f"h1_{m}", name="h1")
                    for k in (0, 1):
                        nc.tensor.matmul(t_[:], w1_sb[:, k, m, :], s1[k][:],
                                         start=(k == 0), stop=(k == 1))
                    h1.append(t_)
                for m in (0, 1):
                    s, _ = swishT(
                        f"b{m}", h1[m],
                        b1s[:, m:m + 1] if b1s is not None else None,
                        b1u[:, m:m + 1] if b1u is not None else None, pre)
                    s2.append(s)
                for m in (0, 1):
                    t_ = ppre.tile([P, BC], f32, tag=f"h2_{m}", name="h2")
                    for k in (0, 1):
                        nc.tensor.matmul(t_[:], w2_sb[:, k, m, :], s2[k][:],
                                         start=(k == 0), stop=(k == 1))
                    h2.append(t_)
                for m in (0, 1):
                    rt = pre.tile([P, BC], f32, tag=f"r_{m}_{branch}", name="rt")
                    nc.vector.tensor_add(rt[:], hb0[m][:], h2[m][:])
                    if b2u is not None:
                        nc.vector.tensor_scalar_add(rt[:], rt[:],
                                                    b2u[:, m:m + 1])
                    sig = pre.tile([P, BC], f32, tag=f"sig3_{m}", name="sig3")
                    nc.scalar.activation(sig[:], rt[:], AF.Sigmoid, bias=0.0,
                                         scale=BETA)
                    s = pre.tile([P, BC], f32, tag=f"s3_{m}_{branch}", name="s3")
                    nc.vector.tensor_mul(s[:], rt[:], sig[:])
                    s3.append(s)
                return s3

            with (
                tc.tile_pool(name="pre", bufs=2) as pre,
                tc.tile_pool(name="ppre", bufs=1, space="PSUM") as ppre,
                tc.tile_pool(name="ptab", bufs=1, space="PSUM") as ptab,
            ):
                s3k = resmlp_T(wkf_sb, bkfs_sb, bkfu_sb, kw1_sb, kb1s_sb,
                               kb1u_sb, kw2_sb, kb2u_sb, pre, ppre, "k")
                s3v = resmlp_T(wvf_sb, None, None, vw1_sb, None, None,
                               vw2_sb, None, pre, ppre, "v")
                # kqT[g, b] = sum_h s3k[h, b] * woq[h, g] + bq[g]
                pkq = ptab.tile([P, 2, BC], f32, tag="pkq")
                for c in (0, 1):
                    nc.tensor.matmul(pkq[:, c, :], woqk_sb[:, 0, c, :],
                                     s3k[0][:], start=True, stop=False)
                    nc.tensor.matmul(pkq[:, c, :], woqk_sb[:, 1, c, :],
                                     s3k[1][:], start=False, stop=False)
                    nc.tensor.matmul(pkq[:, c, :],
                                     bq_sb[0:1, c * P:(c + 1) * P],
                                     ones1[:], start=False, stop=True)
                nc.vector.tensor_copy(kqT16[:], pkq[:])
                # v16[b, f'] (chunked) = sum_h s3v[h, b] * wov[h, f']
                pv = ptab.tile([BC, 2, P], f32, tag="pv")
                for c in (0, 1):
                    for k in (0, 1):
                        nc.tensor.matmul(pv[:, c, :], s3v[k][:],
                                         wovv_sb[:, k, c, :],
                                         start=(k == 0), stop=(k == 1))
                nc.vector.tensor_copy(v16[:], pv[:])

            with (
                tc.tile_pool(name="pdot", bufs=2, space="PSUM") as pd_pool,
                tc.tile_pool(name="pout", bufs=2, space="PSUM") as po_pool,
            ):
                S_tiles = [None] * nsup
                r_tiles = [None] * nsup

                def pass2(s):
                    # S[s] *= r (all molecules of super s closed by now)
                    St = S_tiles[s]
                    rt = r_tiles[min(s + 1, nsup - 1)]
                    nc.vector.tensor_scalar_mul(St[:], St[:], rt[:])
                    out16 = op.tile([P, 2, SUP], f16, tag="out16")
                    for c in (0, 1):
                        po = po_pool.tile([P, 2, HB], f32, tag="po")
                        for b in (0, 1):
                            nc.tensor.matmul(
                                po[:, b, :], v16[:, c, :],
                                St[:, b * HB:(b + 1) * HB],
                                start=True, stop=True)
                            dst = out16[:, c, b * HB:(b + 1) * HB]
                            if c == 1 and b == 1:
                                nc.vector.tensor_copy(dst, po[:, b, :])
                            else:
                                nc.scalar.activation(dst, po[:, b, :], AF.Copy)
                    nc.sync.dma_start(out=ov[:, s, :, :], in_=out16[:])

                x_tiles = [None] * nsup
                m_tiles = [None] * nsup

                def fetch(s):
                    cols = slice(s * SUP, (s + 1) * SUP)
                    x16 = xp.tile([P, 2, SUP], f16, tag="x16")
                    nc.sync.dma_start(out=x16[:], in_=xv[:, s, :, :])
                    sr = mp.tile([1, SUP], f16, tag="sr")
                    nc.sync.dma_start(out=sr[:], in_=mk_h[:, cols])
                    mk = mp.tile([BC, SUP], f16, tag="mk")
                    nc.gpsimd.partition_broadcast(mk[:], sr[:])
                    nc.vector.tensor_scalar(mk[:], mk[:], iota_sb[:], None,
                                            ALU.is_equal)
                    x_tiles[s], m_tiles[s] = x16, mk

                for s0 in range(min(2, nsup)):
                    fetch(s0)
                for s in range(nsup):
                    if s + 2 < nsup:
                        fetch(s + 2)
                    if s >= 3:
                        pass2(s - 3)
                    x16, mk = x_tiles[s], m_tiles[s]

                    pd = pd_pool.tile([BC, 2, HB], f32, tag="pd")
                    for c in (0, 1):
                        for b in (0, 1):
                            nc.tensor.matmul(
                                pd[:, b, :], kqT16[:, c, :],
                                x16[:, c, b * HB:(b + 1) * HB],
                                start=(c == 0), stop=(c == 1),
                                skip_group_check=True)
                    e16 = ep.tile([BC, SUP], f16, tag="e16")
                    nc.scalar.activation(
                        e16[:].rearrange("p (b j) -> p b j", b=2), pd[:],
                        AF.Exp, bias=0.0, scale=INV_SQRT_F)
                    St = sp_.tile([BC, SUP], f16, tag="St")
                    part = rp.tile([BC, 1], f32, tag="part")
                    nc.vector.scalar_tensor_tensor(
                        St[:], e16[:], 1.0, mk[:], ALU.mult, ALU.mult,
                        accum_out=part[:])
                    S_tiles[s] = St
                    nc.vector.tensor_add(anorm_run[:], anorm_run[:], part[:])
                    rt = rp.tile([BC, 1], f32, tag="rt")
                    nc.vector.tensor_scalar_add(rt[:], anorm_run[:], EPS)
                    nc.vector.reciprocal(rt[:], rt[:])
                    r_tiles[s] = rt
                pass2(nsup - 3)
                pass2(nsup - 2)
                pass2(nsup - 1)

    nc.compile()
    return nc


def _prep_host(x, E, batch_seg, Wq, Wkf, bkf, Wvf, kW1, kb1, kW2, kb2, kWo,
               kbo, vW1, vW2, vWo):
    f32 = np.float32
    import ml_dtypes
    f16 = ml_dtypes.bfloat16
    bs = np.asarray(batch_seg).astype(np.int64)
    x = np.asarray(x, dtype=f32)
    N = x.shape[0]
    core_bounds = np.searchsorted(bs, np.arange(NCORES + 1) * BC, side="left")
    NCmax = int(np.max(np.diff(core_bounds)))
    nsup = max(1, -(-NCmax // SUP))
    NCpad = nsup * SUP

    xts, mks, evs = [], [], []
    E32 = np.asarray(E, dtype=f32)
    for c in range(NCORES):
        n0, n1 = core_bounds[c], core_bounds[c + 1]
        nc_ = n1 - n0
        xt = np.zeros((2 * P, NCpad), dtype=f16)
        xt[:, :nc_] = x[n0:n1].T.astype(f16)
        # interleave: [c_chunk*128+p, s*SUP+j] -> [p, s*(2*SUP)+c_chunk*SUP+j]
        xt = np.ascontiguousarray(
            xt.reshape(2, P, nsup, SUP).transpose(1, 2, 0, 3).reshape(P, -1))
        # seg-id row (local molecule ids, -1 on padding); mask built on device
        sr = np.full((1, NCpad), -1.0, dtype=f16)
        sr[0, :nc_] = (bs[n0:n1] - c * BC).astype(f16)
        xts.append(xt)
        mks.append(sr)
        evs.append(np.ascontiguousarray(E32[c * BC:(c + 1) * BC].reshape(1, BC)))

    def pack_w(W):
        A = np.asarray(W, dtype=f32)
        return np.ascontiguousarray(A.reshape(2, P, 2, P).transpose(3, 2, 0, 1))

    def pack_hw(M):
        # M [F(h), F(g)] -> [P(h'), k(h-half), c(g-half), P(g')]
        return np.ascontiguousarray(
            M.reshape(2, P, 2, P).transpose(1, 0, 2, 3))

    def pack_b(v, scale):
        a = (np.asarray(v, dtype=f32) * f32(scale)).astype(f32)
        return np.ascontiguousarray(a.reshape(2, P).T)

    Wq_, kWo_, vWo_ = (np.asarray(a, dtype=f32) for a in (Wq, kWo, vWo))
    woq = (kWo_.T @ Wq_).astype(f32)   # [h, g]
    wov = vWo_.T.astype(f32)           # [h, f]
    weights = dict(
        wkf=np.ascontiguousarray(np.asarray(Wkf, dtype=f32).reshape(F)[None, :]),
        wvf=np.ascontiguousarray(np.asarray(Wvf, dtype=f32).reshape(F)[None, :]),
        kw1=pack_w(kW1), kw2=pack_w(kW2),
        vw1=pack_w(vW1), vw2=pack_w(vW2),
        woqk=pack_hw(woq), wovv=pack_hw(wov),
        bq=np.ascontiguousarray(
            (np.asarray(kbo, dtype=f32) @ Wq_).reshape(1, F)),
        bkfs=pack_b(bkf, BETA), bkfu=pack_b(bkf, 1.0),
        kb1s=pack_b(kb1, BETA), kb1u=pack_b(kb1, 1.0),
        kb2u=pack_b(kb2, 1.0),
        iota=np.arange(BC, dtype=f32).reshape(BC, 1),
    )
    return nsup, xts, mks, evs, weights, core_bounds


_CACHE = {}
LAST_RESULT = None


def kernel(x, E, num_batch, batch_seg, Wq, Wkf, bkf, Wvf, kW1, kb1, kW2, kb2,
           kWo, kbo, vW1, vW2, vWo, **_ignored):
    from concourse.bass_utils import run_bass_kernel_spmd

    nsup, xts, mks, evs, weights, core_bounds = _prep_host(
        x, E, batch_seg, Wq, Wkf, bkf, Wvf, kW1, kb1, kW2, kb2, kWo, kbo,
        vW1, vW2, vWo)

    if nsup not in _CACHE:
        _CACHE[nsup] = _build_program(nsup)
    nc = _CACHE[nsup]

    in_maps = [
        dict(weights, x=xts[c], mk=mks[c], ev=evs[c])
        for c in range(NCORES)
    ]
    res = run_bass_kernel_spmd(nc, in_maps, core_ids=list(range(NCORES)))
    global LAST_RESULT
    LAST_RESULT = res

    NCpad = nsup * SUP
    N = np.asarray(x).shape[0]
    out = np.empty((N, F), dtype=np.float32)
    for c in range(NCORES):
        n0, n1 = core_bounds[c], core_bounds[c + 1]
        o = np.asarray(res.results[c]["out"])
        # [p, s*(2*SUP)+cc*SUP+j] -> [cc*128+p, s*SUP+j]
        oT = o.reshape(P, nsup, 2, SUP).transpose(2, 0, 1, 3).reshape(F, NCpad)
        out[n0:n1] = oT[:, :n1 - n0].T.astype(np.float32)
    return out


# revision 20
# speedup vs baseline: 1.0482x; 1.0482x over previous
"""Trainium2 Bass kernel for NonlinearElectronicEmbedding (segment softmax).

Design ("T2", transposed / padding-free):
  - 512 molecules -> 64 consecutive molecules per core (8 cores). Atoms of
    a core's molecules form one contiguous run (batch_seg sorted); x is
    shipped TRANSPOSED (features on partitions, atoms on the free axis) in
    fp16, so there is no 128-atom padding at all.
  - Prelude computes the k/v tables from E via the ResidualMLPs in
    transposed layout (features on partitions), fusing Wq and kbo@Wq into
    the k-table:  dot(a) = x(a) . (k_mol @ Wq)[seg(a)].
  - Main loop over "supers" of 1024 atoms:
      dots  = kqT^T @ xT           (PE, all 64 molecules at once, fp16)
      e     = exp(dots/16)         (ACT, PSUM->SBUF fp16)
      S     = e * mask, partial = rowsum(S)   (DVE stt fused accum)
      anorm += partial; r = 1/(anorm+eps)     (tiny DVE)
      S[s-1] *= r  (per-partition scalar; every molecule of super s-1 is
                    closed by the end of super s since molecules < 1024)
      outT[s-1] = v16^T @ S[s-1]   (PE outer product, K=64)
      copy PSUM->SBUF fp16 (split ACT/DVE), DMA out.
  - mask is a host-built fp16 0/1 band matrix [64, NCpad] (bs sorted ->
    band). Garbage dot rows (wrong molecules) are zeroed by it; softmax
    shift invariance makes the seg_max pass unnecessary (args bounded).
  - Host does only layout work: transpose+fp16 cast in, transpose+fp32
    cast out.
HBM traffic/core ~ 26+6+26 MB (x + mask + out, fp16) -> memory roofline.
"""

import numpy as np

F = 256
B = 512
NCORES = 8
BC = B // NCORES  # molecules per core
P = 128
SUP = 1024        # atoms per super-group (2 PSUM banks of dots)
HB = SUP // 2     # 512, one PSUM bank
BETA = 1.702
EPS = 1e-8
INV_SQRT_F = 1.0 / 16.0


def _build_program(nsup):
    import concourse.bacc as bacc
    import concourse.mybir as mybir
    import concourse.tile as tile

    dt = mybir.dt
    f32 = dt.float32
    f16 = dt.bfloat16
    AF = mybir.ActivationFunctionType
    ALU = mybir.AluOpType

    NCpad = nsup * SUP

    nc = bacc.Bacc(trn_type="TRN2")

    f8 = dt.float8e4
    x_h = nc.dram_tensor("x", [P, nsup * 2 * SUP], f16, kind="ExternalInput")
    mk_h = nc.dram_tensor("mk", [BC, NCpad], f8, kind="ExternalInput")
    ev_h = nc.dram_tensor("ev", [1, BC], f32, kind="ExternalInput")
    wkf_h = nc.dram_tensor("wkf", [1, F], f32, kind="ExternalInput")
    wvf_h = nc.dram_tensor("wvf", [1, F], f32, kind="ExternalInput")
    kw1_h = nc.dram_tensor("kw1", [P, 2, 2, P], f32, kind="ExternalInput")
    kw2_h = nc.dram_tensor("kw2", [P, 2, 2, P], f32, kind="ExternalInput")
    vw1_h = nc.dram_tensor("vw1", [P, 2, 2, P], f32, kind="ExternalInput")
    vw2_h = nc.dram_tensor("vw2", [P, 2, 2, P], f32, kind="ExternalInput")
    woqk_h = nc.dram_tensor("woqk", [P, 2, 2, P], f32, kind="ExternalInput")
    wovv_h = nc.dram_tensor("wovv", [P, 2, 2, P], f32, kind="ExternalInput")
    bq_h = nc.dram_tensor("bq", [1, F], f32, kind="ExternalInput")
    # biases: [P, 2] chunked; *_s pre-multiplied by BETA, *_u raw
    bkfs_h = nc.dram_tensor("bkfs", [P, 2], f32, kind="ExternalInput")
    bkfu_h = nc.dram_tensor("bkfu", [P, 2], f32, kind="ExternalInput")
    kb1s_h = nc.dram_tensor("kb1s", [P, 2], f32, kind="ExternalInput")
    kb1u_h = nc.dram_tensor("kb1u", [P, 2], f32, kind="ExternalInput")
    kb2u_h = nc.dram_tensor("kb2u", [P, 2], f32, kind="ExternalInput")
    out_h = nc.dram_tensor("out", [P, nsup * 2 * SUP], f16,
                           kind="ExternalOutput")

    # per-super interleaved layout: row p holds [s][c][j] so one DMA moves
    # 4KB contiguous per partition per super
    xv = x_h[:].rearrange("p (s c j) -> p s c j", s=nsup, c=2)
    ov = out_h[:].rearrange("p (s c j) -> p s c j", s=nsup, c=2)

    with tile.TileContext(nc) as tc:
        with (
            tc.tile_pool(name="singles", bufs=1) as sg,
            tc.tile_pool(name="xpool", bufs=4) as xp,
            tc.tile_pool(name="mpool", bufs=4) as mp,
            tc.tile_pool(name="epool", bufs=2) as ep,
            tc.tile_pool(name="spool", bufs=5) as sp_,
            tc.tile_pool(name="opool", bufs=4) as op,
            tc.tile_pool(name="rpool", bufs=5) as rp,
        ):
            def load(name, h, shape):
                t_ = sg.tile(shape, f32, tag=name, name=name)
                nc.sync.dma_start(out=t_[:], in_=h[:])
                return t_

            ev_sb = load("ev", ev_h, [1, BC])
            wkf_sb = load("wkf", wkf_h, [1, F])
            wvf_sb = load("wvf", wvf_h, [1, F])
            kw1_sb = load("kw1", kw1_h, [P, 2, 2, P])
            kw2_sb = load("kw2", kw2_h, [P, 2, 2, P])
            vw1_sb = load("vw1", vw1_h, [P, 2, 2, P])
            vw2_sb = load("vw2", vw2_h, [P, 2, 2, P])
            woqk_sb = load("woqk", woqk_h, [P, 2, 2, P])
            wovv_sb = load("wovv", wovv_h, [P, 2, 2, P])
            bq_sb = load("bq", bq_h, [1, F])
            bkfs_sb = load("bkfs", bkfs_h, [P, 2])
            bkfu_sb = load("bkfu", bkfu_h, [P, 2])
            kb1s_sb = load("kb1s", kb1s_h, [P, 2])
            kb1u_sb = load("kb1u", kb1u_h, [P, 2])
            kb2u_sb = load("kb2u", kb2u_h, [P, 2])

            ones1 = sg.tile([1, BC], f32)
            nc.vector.memset(ones1[:], 1.0)

            kqT16 = sg.tile([P, 2, BC], f16)   # kqT16[f', c, b]
            v16 = sg.tile([BC, 2, P], f16)     # v16[b, c, f']
            anorm_run = sg.tile([BC, 1], f32)
            nc.vector.memset(anorm_run[:], 0.0)

            # ---- prelude: ResidualMLP in transposed layout ----
            # swish(y) = y * sigmoid(BETA*y); h_psum holds y - b.
            def swishT(c, h_psum, bs_ap, bu_ap, pre, keep_hb=False):
                sig = pre.tile([P, BC], f32, tag=f"sig_{c}", name="sig")
                nc.scalar.activation(sig[:], h_psum[:], AF.Sigmoid,
                                     bias=bs_ap if bs_ap is not None else 0.0,
                                     scale=BETA)
                if bu_ap is not None:
                    hb = pre.tile([P, BC], f32, tag=f"hb_{c}", name="hb")
                    nc.vector.tensor_scalar_add(hb[:], h_psum[:], bu_ap)
                elif keep_hb:
                    hb = pre.tile([P, BC], f32, tag=f"hb_{c}", name="hb")
                    nc.vector.tensor_copy(hb[:], h_psum[:])
                else:
                    hb = h_psum
                s = pre.tile([P, BC], f32, tag=f"s_{c}", name="s")
                nc.vector.tensor_mul(s[:], hb[:], sig[:])
                return (s, hb) if keep_hb else (s, None)

            def resmlp_T(wf_sb, b0s, b0u, w1_sb, b1s, b1u, w2_sb, b2u,
                         pre, ppre, branch):
                h0, s1, h1, s2, h2, s3, hb0 = [], [], [], [], [], [], []
                for c in (0, 1):
                    t_ = ppre.tile([P, BC], f32, tag=f"h0_{c}", name="h0")
                    nc.tensor.matmul(t_[:], wf_sb[0:1, c * P:(c + 1) * P],
                                     ev_sb[:], start=True, stop=True)
                    h0.append(t_)
                for c in (0, 1):
                    s, hb = swishT(
                        f"a{c}", h0[c],
                        b0s[:, c:c + 1] if b0s is not None else None,
                        b0u[:, c:c + 1] if b0u is not None else None,
                        pre, keep_hb=True)
                    s1.append(s)
                    hb0.append(hb if hb is not None else h0[c])
                for m in (0, 1):
                    t_ = ppre.tile([P, BC], f32, tag=f"h1_{m}", name="h1")
                    for k in (0, 1):
                        nc.tensor.matmul(t_[:], w1_sb[:, k, m, :], s1[k][:],
                                         start=(k == 0), stop=(k == 1))
                    h1.append(t_)
                for m in (0, 1):
                    s, _ = swishT(
                        f"b{m}", h1[m],
                        b1s[:, m:m + 1] if b1s is not None else None,
                        b1u[:, m:m + 1] if b1u is not None else None, pre)
                    s2.append(s)
                for m in (0, 1):
                    t_ = ppre.tile([P, BC], f32, tag=f"h2_{m}", name="h2")
                    for k in (0, 1):
                        nc.tensor.matmul(t_[:], w2_sb[:, k, m, :], s2[k][:],
                                         start=(k == 0), stop=(k == 1))
                    h2.append(t_)
                for m in (0, 1):
                    rt = pre.tile([P, BC], f32, tag=f"r_{m}_{branch}", name="rt")
                    nc.vector.tensor_add(rt[:], hb0[m][:], h2[m][:])
                    if b2u is not None:
                        nc.vector.tensor_scalar_add(rt[:], rt[:],
                                                    b2u[:, m:m + 1])
                    sig = pre.tile([P, BC], f32, tag=f"sig3_{m}", name="sig3")
                    nc.scalar.activation(sig[:], rt[:], AF.Sigmoid, bias=0.0,
                                         scale=BETA)
                    s = pre.tile([P, BC], f32, tag=f"s3_{m}_{branch}", name="s3")
                    nc.vector.tensor_mul(s[:], rt[:], sig[:])
                    s3.append(s)
                return s3

            with (
                tc.tile_pool(name="pre", bufs=2) as pre,
                tc.tile_pool(name="ppre", bufs=1, space="PSUM") as ppre,
                tc.tile_pool(name="ptab", bufs=1, space="PSUM") as ptab,
            ):
                s3k = resmlp_T(wkf_sb, bkfs_sb, bkfu_sb, kw1_sb, kb1s_sb,
                               kb1u_sb, kw2_sb, kb2u_sb, pre, ppre, "k")
                s3v = resmlp_T(wvf_sb, None, None, vw1_sb, None, None,
                               vw2_sb, None, pre, ppre, "v")
                # kqT[g, b] = sum_h s3k[h, b] * woq[h, g] + bq[g]
                pkq = ptab.tile([P, 2, BC], f32, tag="pkq")
                for c in (0, 1):
                    nc.tensor.matmul(pkq[:, c, :], woqk_sb[:, 0, c, :],
                                     s3k[0][:], start=True, stop=False)
                    nc.tensor.matmul(pkq[:, c, :], woqk_sb[:, 1, c, :],
                                     s3k[1][:], start=False, stop=False)
                    nc.tensor.matmul(pkq[:, c, :],
                                     bq_sb[0:1, c * P:(c + 1) * P],
                                     ones1[:], start=False, stop=True)
                nc.vector.tensor_copy(kqT16[:], pkq[:])
                # v16[b, f'] (chunked) = sum_h s3v[h, b] * wov[h, f']
                pv = ptab.tile([BC, 2, P], f32, tag="pv")
                for c in (0, 1):
                    for k in (0, 1):
                        nc.tensor.matmul(pv[:, c, :], s3v[k][:],
                                         wovv_sb[:, k, c, :],
                                         start=(k == 0), stop=(k == 1))
                nc.vector.tensor_copy(v16[:], pv[:])

            with (
                tc.tile_pool(name="pdot", bufs=2, space="PSUM") as pd_pool,
                tc.tile_pool(name="pout", bufs=2, space="PSUM") as po_pool,
            ):
                S_tiles = [None] * nsup
                r_tiles = [None] * nsup

                def pass2(s):
                    # S[s] *= r (all molecules of super s closed by now)
                    St = S_tiles[s]
                    rt = r_tiles[min(s + 1, nsup - 1)]
                    nc.vector.tensor_scalar_mul(St[:], St[:], rt[:])
                    out16 = op.tile([P, 2, SUP], f16, tag="out16")
                    for c in (0, 1):
                        po = po_pool.tile([P, 2, HB], f32, tag="po")
                        for b in (0, 1):
                            nc.tensor.matmul(
                                po[:, b, :], v16[:, c, :],
                                St[:, b * HB:(b + 1) * HB],
                                start=True, stop=True)
                            dst = out16[:, c, b * HB:(b + 1) * HB]
                            if c == 1 and b == 1:
                                nc.vector.tensor_copy(dst, po[:, b, :])
                            else:
                                nc.scalar.activation(dst, po[:, b, :], AF.Copy)
                    nc.sync.dma_start(out=ov[:, s, :, :], in_=out16[:])

                x_tiles = [None] * nsup
                m_tiles = [None] * nsup

                def fetch(s):
                    cols = slice(s * SUP, (s + 1) * SUP)
                    x16 = xp.tile([P, 2, SUP], f16, tag="x16")
                    nc.sync.dma_start(out=x16[:], in_=xv[:, s, :, :])
                    mk = mp.tile([BC, SUP], f8, tag="mk")
                    nc.sync.dma_start(out=mk[:], in_=mk_h[:, cols])
                    x_tiles[s], m_tiles[s] = x16, mk

                for s0 in range(min(2, nsup)):
                    fetch(s0)
                for s in range(nsup):
                    if s + 2 < nsup:
                        fetch(s + 2)
                    if s >= 3:
                        pass2(s - 3)
                    x16, mk = x_tiles[s], m_tiles[s]

                    pd = pd_pool.tile([BC, 2, HB], f32, tag="pd")
                    for c in (0, 1):
                        for b in (0, 1):
                            nc.tensor.matmul(
                                pd[:, b, :], kqT16[:, c, :],
                                x16[:, c, b * HB:(b + 1) * HB],
                                start=(c == 0), stop=(c == 1),
                                skip_group_check=True)
                    e16 = ep.tile([BC, SUP], f16, tag="e16")
                    nc.scalar.activation(
                        e16[:].rearrange("p (b j) -> p b j", b=2), pd[:],
                        AF.Exp, bias=0.0, scale=INV_SQRT_F)
                    St = sp_.tile([BC, SUP], f16, tag="St")
                    part = rp.tile([BC, 1], f32, tag="part")
                    nc.vector.scalar_tensor_tensor(
                        St[:], e16[:], 1.0, mk[:], ALU.mult, ALU.mult,
                        accum_out=part[:])
                    S_tiles[s] = St
                    nc.vector.tensor_add(anorm_run[:], anorm_run[:], part[:])
                    rt = rp.tile([BC, 1], f32, tag="rt")
                    nc.vector.tensor_scalar_add(rt[:], anorm_run[:], EPS)
                    nc.vector.reciprocal(rt[:], rt[:])
                    r_tiles[s] = rt
                pass2(nsup - 3)
                pass2(nsup - 2)
                pass2(nsup - 1)

    nc.compile()
    return nc


def _prep_host(x, E, batch_seg, Wq, Wkf, bkf, Wvf, kW1, kb1, kW2, kb2, kWo,
               kbo, vW1, vW2, vWo):
    f32 = np.float32
    import ml_dtypes
    f16 = ml_dtypes.bfloat16
    bs = np.asarray(batch_seg).astype(np.int64)
    x = np.asarray(x, dtype=f32)
    N = x.shape[0]
    core_bounds = np.searchsorted(bs, np.arange(NCORES + 1) * BC, side="left")
    NCmax = int(np.max(np.diff(core_bounds)))
    nsup = max(1, -(-NCmax // SUP))
    NCpad = nsup * SUP

    xts, mks, evs = [], [], []
    E32 = np.asarray(E, dtype=f32)
    for c in range(NCORES):
        n0, n1 = core_bounds[c], core_bounds[c + 1]
        nc_ = n1 - n0
        xt = np.zeros((2 * P, NCpad), dtype=f16)
        xt[:, :nc_] = x[n0:n1].T.astype(f16)
        # interleave: [c_chunk*128+p, s*SUP+j] -> [p, s*(2*SUP)+c_chunk*SUP+j]
        xt = np.ascontiguousarray(
            xt.reshape(2, P, nsup, SUP).transpose(1, 2, 0, 3).reshape(P, -1))
        f8 = ml_dtypes.float8_e4m3fn
        mk = np.zeros((BC, NCpad), dtype=f8)
        mk[:, :nc_] = (bs[n0:n1][None, :]
                       == (np.arange(BC) + c * BC)[:, None]).astype(f8)
        xts.append(xt)
        mks.append(mk)
        evs.append(np.ascontiguousarray(E32[c * BC:(c + 1) * BC].reshape(1, BC)))

    def pack_w(W):
        A = np.asarray(W, dtype=f32)
        return np.ascontiguousarray(A.reshape(2, P, 2, P).transpose(3, 2, 0, 1))

    def pack_hw(M):
        # M [F(h), F(g)] -> [P(h'), k(h-half), c(g-half), P(g')]
        return np.ascontiguousarray(
            M.reshape(2, P, 2, P).transpose(1, 0, 2, 3))

    def pack_b(v, scale):
        a = (np.asarray(v, dtype=f32) * f32(scale)).astype(f32)
        return np.ascontiguousarray(a.reshape(2, P).T)

    Wq_, kWo_, vWo_ = (np.asarray(a, dtype=f32) for a in (Wq, kWo, vWo))
    woq = (kWo_.T @ Wq_).astype(f32)   # [h, g]
    wov = vWo_.T.astype(f32)           # [h, f]
    weights = dict(
        wkf=np.ascontiguousarray(np.asarray(Wkf, dtype=f32).reshape(F)[None, :]),
        wvf=np.ascontiguousarray(np.asarray(Wvf, dtype=f32).reshape(F)[None, :]),
        kw1=pack_w(kW1), kw2=pack_w(kW2),
        vw1=pack_w(vW1), vw2=pack_w(vW2),
        woqk=pack_hw(woq), wovv=pack_hw(wov),
        bq=np.ascontiguousarray(
            (np.asarray(kbo, dtype=f32) @ Wq_).reshape(1, F)),
        bkfs=pack_b(bkf, BETA), bkfu=pack_b(bkf, 1.0),
        kb1s=pack_b(kb1, BETA), kb1u=pack_b(kb1, 1.0),
        kb2u=pack_b(kb2, 1.0),
    )
    return nsup, xts, mks, evs, weights, core_bounds


_CACHE = {}
LAST_RESULT = None


def kernel(x, E, num_batch, batch_seg, Wq, Wkf, bkf, Wvf, kW1, kb1, kW2, kb2,
           kWo, kbo, vW1, vW2, vWo, **_ignored):
    from concourse.bass_utils import run_bass_kernel_spmd

    nsup, xts, mks, evs, weights, core_bounds = _prep_host(
        x, E, batch_seg, Wq, Wkf, bkf, Wvf, kW1, kb1, kW2, kb2, kWo, kbo,
        vW1, vW2, vWo)

    if nsup not in _CACHE:
        _CACHE[nsup] = _build_program(nsup)
    nc = _CACHE[nsup]

    in_maps = [
        dict(weights, x=xts[c], mk=mks[c], ev=evs[c])
        for c in range(NCORES)
    ]
    res = run_bass_kernel_spmd(nc, in_maps, core_ids=list(range(NCORES)))
    global LAST_RESULT
    LAST_RESULT = res

    NCpad = nsup * SUP
    N = np.asarray(x).shape[0]
    out = np.empty((N, F), dtype=np.float32)
    for c in range(NCORES):
        n0, n1 = core_bounds[c], core_bounds[c + 1]
        o = np.asarray(res.results[c]["out"])
        # [p, s*(2*SUP)+cc*SUP+j] -> [cc*128+p, s*SUP+j]
        oT = o.reshape(P, nsup, 2, SUP).transpose(2, 0, 1, 3).reshape(F, NCpad)
        out[n0:n1] = oT[:, :n1 - n0].T.astype(np.float32)
    return out


# revision 21
# speedup vs baseline: 1.2408x; 1.1837x over previous
"""Trainium2 Bass kernel for NonlinearElectronicEmbedding (segment softmax).

Design ("T2", transposed / padding-free):
  - 512 molecules -> 64 consecutive molecules per core (8 cores). Atoms of
    a core's molecules form one contiguous run (batch_seg sorted); x is
    shipped TRANSPOSED (features on partitions, atoms on the free axis) in
    fp16, so there is no 128-atom padding at all.
  - Prelude computes the k/v tables from E via the ResidualMLPs in
    transposed layout (features on partitions), fusing Wq and kbo@Wq into
    the k-table:  dot(a) = x(a) . (k_mol @ Wq)[seg(a)].
  - Main loop over "supers" of 1024 atoms:
      dots  = kqT^T @ xT           (PE, all 64 molecules at once, fp16)
      e     = exp(dots/16)         (ACT, PSUM->SBUF fp16)
      S     = e * mask, partial = rowsum(S)   (DVE stt fused accum)
      anorm += partial; r = 1/(anorm+eps)     (tiny DVE)
      S[s-1] *= r  (per-partition scalar; every molecule of super s-1 is
                    closed by the end of super s since molecules < 1024)
      outT[s-1] = v16^T @ S[s-1]   (PE outer product, K=64)
      copy PSUM->SBUF fp16 (split ACT/DVE), DMA out.
  - mask is a host-built fp16 0/1 band matrix [64, NCpad] (bs sorted ->
    band). Garbage dot rows (wrong molecules) are zeroed by it; softmax
    shift invariance makes the seg_max pass unnecessary (args bounded).
  - Host does only layout work: transpose+fp16 cast in, transpose+fp32
    cast out.
HBM traffic/core ~ 26+6+26 MB (x + mask + out, fp16) -> memory roofline.
"""

import numpy as np

F = 256
B = 512
NCORES = 8
BC = B // NCORES  # molecules per core
P = 128
SUP = 1024        # atoms per super-group (2 PSUM banks of dots)
HB = SUP // 2     # 512, one PSUM bank
BETA = 1.702
EPS = 1e-8
INV_SQRT_F = 1.0 / 16.0


def _build_program(nsup):
    import concourse.bacc as bacc
    import concourse.mybir as mybir
    import concourse.tile as tile

    dt = mybir.dt
    f32 = dt.float32
    f16 = dt.bfloat16
    AF = mybir.ActivationFunctionType
    ALU = mybir.AluOpType

    NCpad = nsup * SUP

    nc = bacc.Bacc(trn_type="TRN2")

    f8 = dt.float8e4
    x_h = nc.dram_tensor("x", [P, nsup * 2 * SUP], f16, kind="ExternalInput")
    mk_h = nc.dram_tensor("mk", [BC, NCpad], f8, kind="ExternalInput")
    ev_h = nc.dram_tensor("ev", [1, BC], f32, kind="ExternalInput")
    wkf_h = nc.dram_tensor("wkf", [1, F], f32, kind="ExternalInput")
    wvf_h = nc.dram_tensor("wvf", [1, F], f32, kind="ExternalInput")
    kw1_h = nc.dram_tensor("kw1", [P, 2, 2, P], f32, kind="ExternalInput")
    kw2_h = nc.dram_tensor("kw2", [P, 2, 2, P], f32, kind="ExternalInput")
    vw1_h = nc.dram_tensor("vw1", [P, 2, 2, P], f32, kind="ExternalInput")
    vw2_h = nc.dram_tensor("vw2", [P, 2, 2, P], f32, kind="ExternalInput")
    woqk_h = nc.dram_tensor("woqk", [P, 2, 2, P], f32, kind="ExternalInput")
    wovv_h = nc.dram_tensor("wovv", [P, 2, 2, P], f32, kind="ExternalInput")
    bq_h = nc.dram_tensor("bq", [1, F], f32, kind="ExternalInput")
    # biases: [P, 2] chunked; *_s pre-multiplied by BETA, *_u raw
    bkfs_h = nc.dram_tensor("bkfs", [P, 2], f32, kind="ExternalInput")
    bkfu_h = nc.dram_tensor("bkfu", [P, 2], f32, kind="ExternalInput")
    kb1s_h = nc.dram_tensor("kb1s", [P, 2], f32, kind="ExternalInput")
    kb1u_h = nc.dram_tensor("kb1u", [P, 2], f32, kind="ExternalInput")
    kb2u_h = nc.dram_tensor("kb2u", [P, 2], f32, kind="ExternalInput")
    out_h = nc.dram_tensor("out", [P, nsup * 2 * SUP], f16,
                           kind="ExternalOutput")

    # per-super interleaved layout: row p holds [s][c][j] so one DMA moves
    # 4KB contiguous per partition per super
    xv = x_h[:].rearrange("p (s c j) -> p s c j", s=nsup, c=2)
    ov = out_h[:].rearrange("p (s c j) -> p s c j", s=nsup, c=2)

    with tile.TileContext(nc) as tc:
        with (
            tc.tile_pool(name="singles", bufs=1) as sg,
            tc.tile_pool(name="xpool", bufs=4) as xp,
            tc.tile_pool(name="mpool", bufs=4) as mp,
            tc.tile_pool(name="epool", bufs=2) as ep,
            tc.tile_pool(name="spool", bufs=5) as sp_,
            tc.tile_pool(name="opool", bufs=4) as op,
            tc.tile_pool(name="rpool", bufs=5) as rp,
        ):
            def load(name, h, shape):
                t_ = sg.tile(shape, f32, tag=name, name=name)
                nc.sync.dma_start(out=t_[:], in_=h[:])
                return t_

            ev_sb = load("ev", ev_h, [1, BC])
            wkf_sb = load("wkf", wkf_h, [1, F])
            wvf_sb = load("wvf", wvf_h, [1, F])
            kw1_sb = load("kw1", kw1_h, [P, 2, 2, P])
            kw2_sb = load("kw2", kw2_h, [P, 2, 2, P])
            vw1_sb = load("vw1", vw1_h, [P, 2, 2, P])
            vw2_sb = load("vw2", vw2_h, [P, 2, 2, P])
            woqk_sb = load("woqk", woqk_h, [P, 2, 2, P])
            wovv_sb = load("wovv", wovv_h, [P, 2, 2, P])
            bq_sb = load("bq", bq_h, [1, F])
            bkfs_sb = load("bkfs", bkfs_h, [P, 2])
            bkfu_sb = load("bkfu", bkfu_h, [P, 2])
            kb1s_sb = load("kb1s", kb1s_h, [P, 2])
            kb1u_sb = load("kb1u", kb1u_h, [P, 2])
            kb2u_sb = load("kb2u", kb2u_h, [P, 2])

            ones1 = sg.tile([1, BC], f32)
            nc.vector.memset(ones1[:], 1.0)

            kqT16 = sg.tile([P, 2, BC], f16)   # kqT16[f', c, b]
            v16 = sg.tile([BC, 2, P], f16)     # v16[b, c, f']
            anorm_run = sg.tile([BC, 1], f32)
            nc.vector.memset(anorm_run[:], 0.0)

            # ---- prelude: ResidualMLP in transposed layout ----
            # swish(y) = y * sigmoid(BETA*y); h_psum holds y - b.
            def swishT(c, h_psum, bs_ap, bu_ap, pre, keep_hb=False):
                sig = pre.tile([P, BC], f32, tag=f"sig_{c}", name="sig")
                nc.scalar.activation(sig[:], h_psum[:], AF.Sigmoid,
                                     bias=bs_ap if bs_ap is not None else 0.0,
                                     scale=BETA)
                if bu_ap is not None:
                    hb = pre.tile([P, BC], f32, tag=f"hb_{c}", name="hb")
                    nc.vector.tensor_scalar_add(hb[:], h_psum[:], bu_ap)
                elif keep_hb:
                    hb = pre.tile([P, BC], f32, tag=f"hb_{c}", name="hb")
                    nc.vector.tensor_copy(hb[:], h_psum[:])
                else:
                    hb = h_psum
                s = pre.tile([P, BC], f32, tag=f"s_{c}", name="s")
                nc.vector.tensor_mul(s[:], hb[:], sig[:])
                return (s, hb) if keep_hb else (s, None)

            def resmlp_T(wf_sb, b0s, b0u, w1_sb, b1s, b1u, w2_sb, b2u,
                         pre, ppre, branch):
                h0, s1, h1, s2, h2, s3, hb0 = [], [], [], [], [], [], []
                for c in (0, 1):
                    t_ = ppre.tile([P, BC], f32, tag=f"h0_{c}", name="h0")
                    nc.tensor.matmul(t_[:], wf_sb[0:1, c * P:(c + 1) * P],
                                     ev_sb[:], start=True, stop=True)
                    h0.append(t_)
                for c in (0, 1):
                    s, hb = swishT(
                        f"a{c}", h0[c],
                        b0s[:, c:c + 1] if b0s is not None else None,
                        b0u[:, c:c + 1] if b0u is not None else None,
                        pre, keep_hb=True)
                    s1.append(s)
                    hb0.append(hb if hb is not None else h0[c])
                for m in (0, 1):
                    t_ = ppre.tile([P, BC], f32, tag=f"h1_{m}", name="h1")
                    for k in (0, 1):
                        nc.tensor.matmul(t_[:], w1_sb[:, k, m, :], s1[k][:],
                                         start=(k == 0), stop=(k == 1))
                    h1.append(t_)
                for m in (0, 1):
                    s, _ = swishT(
                        f"b{m}", h1[m],
                        b1s[:, m:m + 1] if b1s is not None else None,
                        b1u[:, m:m + 1] if b1u is not None else None, pre)
                    s2.append(s)
                for m in (0, 1):
                    t_ = ppre.tile([P, BC], f32, tag=f"h2_{m}", name="h2")
                    for k in (0, 1):
                        nc.tensor.matmul(t_[:], w2_sb[:, k, m, :], s2[k][:],
                                         start=(k == 0), stop=(k == 1))
                    h2.append(t_)
                for m in (0, 1):
                    rt = pre.tile([P, BC], f32, tag=f"r_{m}_{branch}", name="rt")
                    nc.vector.tensor_add(rt[:], hb0[m][:], h2[m][:])
                    if b2u is not None:
                        nc.vector.tensor_scalar_add(rt[:], rt[:],
                                                    b2u[:, m:m + 1])
                    sig = pre.tile([P, BC], f32, tag=f"sig3_{m}", name="sig3")
                    nc.scalar.activation(sig[:], rt[:], AF.Sigmoid, bias=0.0,
                                         scale=BETA)
                    s = pre.tile([P, BC], f32, tag=f"s3_{m}_{branch}", name="s3")
                    nc.vector.tensor_mul(s[:], rt[:], sig[:])
                    s3.append(s)
                return s3

            with (
                tc.tile_pool(name="pre", bufs=2) as pre,
                tc.tile_pool(name="ppre", bufs=1, space="PSUM") as ppre,
                tc.tile_pool(name="ptab", bufs=1, space="PSUM") as ptab,
            ):
                s3k = resmlp_T(wkf_sb, bkfs_sb, bkfu_sb, kw1_sb, kb1s_sb,
                               kb1u_sb, kw2_sb, kb2u_sb, pre, ppre, "k")
                s3v = resmlp_T(wvf_sb, None, None, vw1_sb, None, None,
                               vw2_sb, None, pre, ppre, "v")
                # kqT[g, b] = sum_h s3k[h, b] * woq[h, g] + bq[g]
                pkq = ptab.tile([P, 2, BC], f32, tag="pkq")
                for c in (0, 1):
                    nc.tensor.matmul(pkq[:, c, :], woqk_sb[:, 0, c, :],
                                     s3k[0][:], start=True, stop=False)
                    nc.tensor.matmul(pkq[:, c, :], woqk_sb[:, 1, c, :],
                                     s3k[1][:], start=False, stop=False)
                    nc.tensor.matmul(pkq[:, c, :],
                                     bq_sb[0:1, c * P:(c + 1) * P],
                                     ones1[:], start=False, stop=True)
                nc.vector.tensor_copy(kqT16[:], pkq[:])
                # v16[b, f'] (chunked) = sum_h s3v[h, b] * wov[h, f']
                pv = ptab.tile([BC, 2, P], f32, tag="pv")
                for c in (0, 1):
                    for k in (0, 1):
                        nc.tensor.matmul(pv[:, c, :], s3v[k][:],
                                         wovv_sb[:, k, c, :],
                                         start=(k == 0), stop=(k == 1))
                nc.vector.tensor_copy(v16[:], pv[:])

            with (
                tc.tile_pool(name="pdot", bufs=2, space="PSUM") as pd_pool,
                tc.tile_pool(name="pout", bufs=2, space="PSUM") as po_pool,
            ):
                S_tiles = [None] * nsup
                r_tiles = [None] * nsup

                def pass2(s):
                    # S[s] *= r (all molecules of super s closed by now)
                    St = S_tiles[s]
                    rt = r_tiles[min(s + 1, nsup - 1)]
                    nc.vector.tensor_scalar_mul(St[:], St[:], rt[:])
                    out16 = op.tile([P, 2, SUP], f16, tag="out16")
                    for c in (0, 1):
                        po = po_pool.tile([P, 2, HB], f32, tag="po")
                        for b in (0, 1):
                            nc.tensor.matmul(
                                po[:, b, :], v16[:, c, :],
                                St[:, b * HB:(b + 1) * HB],
                                start=True, stop=True)
                        dst = out16[:, c, :].rearrange("p (b j) -> p b j", b=2)
                        nc.scalar.activation(dst, po[:], AF.Copy)
                    nc.sync.dma_start(out=ov[:, s, :, :], in_=out16[:])

                x_tiles = [None] * nsup
                m_tiles = [None] * nsup

                def fetch(s):
                    cols = slice(s * SUP, (s + 1) * SUP)
                    x16 = xp.tile([P, 2, SUP], f16, tag="x16")
                    nc.sync.dma_start(out=x16[:], in_=xv[:, s, :, :])
                    mk = mp.tile([BC, SUP], f8, tag="mk")
                    nc.sync.dma_start(out=mk[:], in_=mk_h[:, cols])
                    x_tiles[s], m_tiles[s] = x16, mk

                for s0 in range(min(2, nsup)):
                    fetch(s0)
                for s in range(nsup):
                    if s + 2 < nsup:
                        fetch(s + 2)
                    if s >= 3:
                        pass2(s - 3)
                    x16, mk = x_tiles[s], m_tiles[s]

                    pd = pd_pool.tile([BC, 2, HB], f32, tag="pd")
                    for c in (0, 1):
                        for b in (0, 1):
                            nc.tensor.matmul(
                                pd[:, b, :], kqT16[:, c, :],
                                x16[:, c, b * HB:(b + 1) * HB],
                                start=(c == 0), stop=(c == 1),
                                skip_group_check=True)
                    e16 = ep.tile([BC, SUP], f16, tag="e16")
                    nc.scalar.activation(
                        e16[:].rearrange("p (b j) -> p b j", b=2), pd[:],
                        AF.Exp, bias=0.0, scale=INV_SQRT_F)
                    St = sp_.tile([BC, SUP], f16, tag="St")
                    part = rp.tile([BC, 1], f32, tag="part")
                    nc.vector.scalar_tensor_tensor(
                        St[:], e16[:], 1.0, mk[:], ALU.mult, ALU.mult,
                        accum_out=part[:])
                    S_tiles[s] = St
                    nc.vector.tensor_add(anorm_run[:], anorm_run[:], part[:])
                    rt = rp.tile([BC, 1], f32, tag="rt")
                    nc.vector.tensor_scalar_add(rt[:], anorm_run[:], EPS)
                    nc.vector.reciprocal(rt[:], rt[:])
                    r_tiles[s] = rt
                pass2(nsup - 3)
                pass2(nsup - 2)
                pass2(nsup - 1)

    nc.compile()
    return nc


def _prep_host(x, E, batch_seg, Wq, Wkf, bkf, Wvf, kW1, kb1, kW2, kb2, kWo,
               kbo, vW1, vW2, vWo):
    f32 = np.float32
    import ml_dtypes
    f16 = ml_dtypes.bfloat16
    bs = np.asarray(batch_seg).astype(np.int64)
    x = np.asarray(x, dtype=f32)
    N = x.shape[0]
    core_bounds = np.searchsorted(bs, np.arange(NCORES + 1) * BC, side="left")
    NCmax = int(np.max(np.diff(core_bounds)))
    nsup = max(1, -(-NCmax // SUP))
    NCpad = nsup * SUP

    xts, mks, evs = [], [], []
    E32 = np.asarray(E, dtype=f32)
    for c in range(NCORES):
        n0, n1 = core_bounds[c], core_bounds[c + 1]
        nc_ = n1 - n0
        xt = np.zeros((2 * P, NCpad), dtype=f16)
        xt[:, :nc_] = x[n0:n1].T.astype(f16)
        # interleave: [c_chunk*128+p, s*SUP+j] -> [p, s*(2*SUP)+c_chunk*SUP+j]
        xt = np.ascontiguousarray(
            xt.reshape(2, P, nsup, SUP).transpose(1, 2, 0, 3).reshape(P, -1))
        f8 = ml_dtypes.float8_e4m3fn
        mk = np.zeros((BC, NCpad), dtype=f8)
        mk[:, :nc_] = (bs[n0:n1][None, :]
                       == (np.arange(BC) + c * BC)[:, None]).astype(f8)
        xts.append(xt)
        mks.append(mk)
        evs.append(np.ascontiguousarray(E32[c * BC:(c + 1) * BC].reshape(1, BC)))

    def pack_w(W):
        A = np.asarray(W, dtype=f32)
        return np.ascontiguousarray(A.reshape(2, P, 2, P).transpose(3, 2, 0, 1))

    def pack_hw(M):
        # M [F(h), F(g)] -> [P(h'), k(h-half), c(g-half), P(g')]
        return np.ascontiguousarray(
            M.reshape(2, P, 2, P).transpose(1, 0, 2, 3))

    def pack_b(v, scale):
        a = (np.asarray(v, dtype=f32) * f32(scale)).astype(f32)
        return np.ascontiguousarray(a.reshape(2, P).T)

    Wq_, kWo_, vWo_ = (np.asarray(a, dtype=f32) for a in (Wq, kWo, vWo))
    woq = (kWo_.T @ Wq_).astype(f32)   # [h, g]
    wov = vWo_.T.astype(f32)           # [h, f]
    weights = dict(
        wkf=np.ascontiguousarray(np.asarray(Wkf, dtype=f32).reshape(F)[None, :]),
        wvf=np.ascontiguousarray(np.asarray(Wvf, dtype=f32).reshape(F)[None, :]),
        kw1=pack_w(kW1), kw2=pack_w(kW2),
        vw1=pack_w(vW1), vw2=pack_w(vW2),
        woqk=pack_hw(woq), wovv=pack_hw(wov),
        bq=np.ascontiguousarray(
            (np.asarray(kbo, dtype=f32) @ Wq_).reshape(1, F)),
        bkfs=pack_b(bkf, BETA), bkfu=pack_b(bkf, 1.0),
        kb1s=pack_b(kb1, BETA), kb1u=pack_b(kb1, 1.0),
        kb2u=pack_b(kb2, 1.0),
    )
    return nsup, xts, mks, evs, weights, core_bounds


_CACHE = {}
LAST_RESULT = None


def kernel(x, E, num_batch, batch_seg, Wq, Wkf, bkf, Wvf, kW1, kb1, kW2, kb2,
           kWo, kbo, vW1, vW2, vWo, **_ignored):
    from concourse.bass_utils import run_bass_kernel_spmd

    nsup, xts, mks, evs, weights, core_bounds = _prep_host(
        x, E, batch_seg, Wq, Wkf, bkf, Wvf, kW1, kb1, kW2, kb2, kWo, kbo,
        vW1, vW2, vWo)

    if nsup not in _CACHE:
        _CACHE[nsup] = _build_program(nsup)
    nc = _CACHE[nsup]

    in_maps = [
        dict(weights, x=xts[c], mk=mks[c], ev=evs[c])
        for c in range(NCORES)
    ]
    res = run_bass_kernel_spmd(nc, in_maps, core_ids=list(range(NCORES)))
    global LAST_RESULT
    LAST_RESULT = res

    NCpad = nsup * SUP
    N = np.asarray(x).shape[0]
    out = np.empty((N, F), dtype=np.float32)
    for c in range(NCORES):
        n0, n1 = core_bounds[c], core_bounds[c + 1]
        o = np.asarray(res.results[c]["out"])
        # [p, s*(2*SUP)+cc*SUP+j] -> [cc*128+p, s*SUP+j]
        oT = o.reshape(P, nsup, 2, SUP).transpose(2, 0, 1, 3).reshape(F, NCpad)
        out[n0:n1] = oT[:, :n1 - n0].T.astype(np.float32)
    return out


# revision 22
# speedup vs baseline: 1.2612x; 1.0165x over previous
"""Trainium2 Bass kernel for NonlinearElectronicEmbedding (segment softmax).

Design ("T2", transposed / padding-free):
  - 512 molecules -> 64 consecutive molecules per core (8 cores). Atoms of
    a core's molecules form one contiguous run (batch_seg sorted); x is
    shipped TRANSPOSED (features on partitions, atoms on the free axis) in
    fp16, so there is no 128-atom padding at all.
  - Prelude computes the k/v tables from E via the ResidualMLPs in
    transposed layout (features on partitions), fusing Wq and kbo@Wq into
    the k-table:  dot(a) = x(a) . (k_mol @ Wq)[seg(a)].
  - Main loop over "supers" of 1024 atoms:
      dots  = kqT^T @ xT           (PE, all 64 molecules at once, fp16)
      e     = exp(dots/16)         (ACT, PSUM->SBUF fp16)
      S     = e * mask, partial = rowsum(S)   (DVE stt fused accum)
      anorm += partial; r = 1/(anorm+eps)     (tiny DVE)
      S[s-1] *= r  (per-partition scalar; every molecule of super s-1 is
                    closed by the end of super s since molecules < 1024)
      outT[s-1] = v16^T @ S[s-1]   (PE outer product, K=64)
      copy PSUM->SBUF fp16 (split ACT/DVE), DMA out.
  - mask is a host-built fp16 0/1 band matrix [64, NCpad] (bs sorted ->
    band). Garbage dot rows (wrong molecules) are zeroed by it; softmax
    shift invariance makes the seg_max pass unnecessary (args bounded).
  - Host does only layout work: transpose+fp16 cast in, transpose+fp32
    cast out.
HBM traffic/core ~ 26+6+26 MB (x + mask + out, fp16) -> memory roofline.
"""

import numpy as np

F = 256
B = 512
NCORES = 8
BC = B // NCORES  # molecules per core
P = 128
SUP = 1024        # atoms per super-group (2 PSUM banks of dots)
HB = SUP // 2     # 512, one PSUM bank
BETA = 1.702
EPS = 1e-8
INV_SQRT_F = 1.0 / 16.0


def _build_program(nsup):
    import concourse.bacc as bacc
    import concourse.mybir as mybir
    import concourse.tile as tile

    dt = mybir.dt
    f32 = dt.float32
    f16 = dt.bfloat16
    AF = mybir.ActivationFunctionType
    ALU = mybir.AluOpType

    NCpad = nsup * SUP

    nc = bacc.Bacc(trn_type="TRN2")

    f8 = dt.float8e4
    x_h = nc.dram_tensor("x", [P, nsup * 2 * SUP], f16, kind="ExternalInput")
    mk_h = nc.dram_tensor("mk", [BC, NCpad], f8, kind="ExternalInput")
    ev_h = nc.dram_tensor("ev", [1, BC], f32, kind="ExternalInput")
    wkf_h = nc.dram_tensor("wkf", [1, F], f32, kind="ExternalInput")
    wvf_h = nc.dram_tensor("wvf", [1, F], f32, kind="ExternalInput")
    kw1_h = nc.dram_tensor("kw1", [P, 2, 2, P], f32, kind="ExternalInput")
    kw2_h = nc.dram_tensor("kw2", [P, 2, 2, P], f32, kind="ExternalInput")
    vw1_h = nc.dram_tensor("vw1", [P, 2, 2, P], f32, kind="ExternalInput")
    vw2_h = nc.dram_tensor("vw2", [P, 2, 2, P], f32, kind="ExternalInput")
    woqk_h = nc.dram_tensor("woqk", [P, 2, 2, P], f32, kind="ExternalInput")
    wovv_h = nc.dram_tensor("wovv", [P, 2, 2, P], f32, kind="ExternalInput")
    bq_h = nc.dram_tensor("bq", [1, F], f32, kind="ExternalInput")
    # biases: [P, 2] chunked; *_s pre-multiplied by BETA, *_u raw
    bkfs_h = nc.dram_tensor("bkfs", [P, 2], f32, kind="ExternalInput")
    bkfu_h = nc.dram_tensor("bkfu", [P, 2], f32, kind="ExternalInput")
    kb1s_h = nc.dram_tensor("kb1s", [P, 2], f32, kind="ExternalInput")
    kb1u_h = nc.dram_tensor("kb1u", [P, 2], f32, kind="ExternalInput")
    kb2u_h = nc.dram_tensor("kb2u", [P, 2], f32, kind="ExternalInput")
    out_h = nc.dram_tensor("out", [P, nsup * 2 * SUP], f16,
                           kind="ExternalOutput")

    # per-super interleaved layout: row p holds [s][c][j] so one DMA moves
    # 4KB contiguous per partition per super
    xv = x_h[:].rearrange("p (s c j) -> p s c j", s=nsup, c=2)
    ov = out_h[:].rearrange("p (s c j) -> p s c j", s=nsup, c=2)

    with tile.TileContext(nc) as tc:
        with (
            tc.tile_pool(name="singles", bufs=1) as sg,
            tc.tile_pool(name="xpool", bufs=4) as xp,
            tc.tile_pool(name="mpool", bufs=4) as mp,
            tc.tile_pool(name="epool", bufs=2) as ep,
            tc.tile_pool(name="spool", bufs=5) as sp_,
            tc.tile_pool(name="opool", bufs=4) as op,
            tc.tile_pool(name="rpool", bufs=5) as rp,
        ):
            # issue the first x/mask fetches before the weight loads so the
            # big input stream starts immediately (SP queue is in-order)
            early = []
            for s0 in (0, 1):
                if s0 >= nsup:
                    break
                x16e = xp.tile([P, 2, SUP], f16, tag="x16", name="x16e")
                nc.sync.dma_start(out=x16e[:], in_=xv[:, s0, :, :])
                mke = mp.tile([BC, SUP], f8, tag="mk", name="mke")
                nc.sync.dma_start(out=mke[:],
                                  in_=mk_h[:, s0 * SUP:(s0 + 1) * SUP])
                early.append((x16e, mke))

            def load(name, h, shape):
                t_ = sg.tile(shape, f32, tag=name, name=name)
                nc.sync.dma_start(out=t_[:], in_=h[:])
                return t_

            ev_sb = load("ev", ev_h, [1, BC])
            wkf_sb = load("wkf", wkf_h, [1, F])
            wvf_sb = load("wvf", wvf_h, [1, F])
            kw1_sb = load("kw1", kw1_h, [P, 2, 2, P])
            kw2_sb = load("kw2", kw2_h, [P, 2, 2, P])
            vw1_sb = load("vw1", vw1_h, [P, 2, 2, P])
            vw2_sb = load("vw2", vw2_h, [P, 2, 2, P])
            woqk_sb = load("woqk", woqk_h, [P, 2, 2, P])
            wovv_sb = load("wovv", wovv_h, [P, 2, 2, P])
            bq_sb = load("bq", bq_h, [1, F])
            bkfs_sb = load("bkfs", bkfs_h, [P, 2])
            bkfu_sb = load("bkfu", bkfu_h, [P, 2])
            kb1s_sb = load("kb1s", kb1s_h, [P, 2])
            kb1u_sb = load("kb1u", kb1u_h, [P, 2])
            kb2u_sb = load("kb2u", kb2u_h, [P, 2])

            ones1 = sg.tile([1, BC], f32)
            nc.vector.memset(ones1[:], 1.0)

            kqT16 = sg.tile([P, 2, BC], f16)   # kqT16[f', c, b]
            v16 = sg.tile([BC, 2, P], f16)     # v16[b, c, f']
            anorm_run = sg.tile([BC, 1], f32)
            nc.vector.memset(anorm_run[:], 0.0)

            # ---- prelude: ResidualMLP in transposed layout ----
            # swish(y) = y * sigmoid(BETA*y); h_psum holds y - b.
            def swishT(c, h_psum, bs_ap, bu_ap, pre, keep_hb=False):
                sig = pre.tile([P, BC], f32, tag=f"sig_{c}", name="sig")
                nc.scalar.activation(sig[:], h_psum[:], AF.Sigmoid,
                                     bias=bs_ap if bs_ap is not None else 0.0,
                                     scale=BETA)
                if bu_ap is not None:
                    hb = pre.tile([P, BC], f32, tag=f"hb_{c}", name="hb")
                    nc.vector.tensor_scalar_add(hb[:], h_psum[:], bu_ap)
                elif keep_hb:
                    hb = pre.tile([P, BC], f32, tag=f"hb_{c}", name="hb")
                    nc.vector.tensor_copy(hb[:], h_psum[:])
                else:
                    hb = h_psum
                s = pre.tile([P, BC], f32, tag=f"s_{c}", name="s")
                nc.vector.tensor_mul(s[:], hb[:], sig[:])
                return (s, hb) if keep_hb else (s, None)

            def resmlp_T(wf_sb, b0s, b0u, w1_sb, b1s, b1u, w2_sb, b2u,
                         pre, ppre, branch):
                h0, s1, h1, s2, h2, s3, hb0 = [], [], [], [], [], [], []
                for c in (0, 1):
                    t_ = ppre.tile([P, BC], f32, tag=f"h0_{c}", name="h0")
                    nc.tensor.matmul(t_[:], wf_sb[0:1, c * P:(c + 1) * P],
                                     ev_sb[:], start=True, stop=True)
                    h0.append(t_)
                for c in (0, 1):
                    s, hb = swishT(
                        f"a{c}", h0[c],
                        b0s[:, c:c + 1] if b0s is not None else None,
                        b0u[:, c:c + 1] if b0u is not None else None,
                        pre, keep_hb=True)
                    s1.append(s)
                    hb0.append(hb if hb is not None else h0[c])
                for m in (0, 1):
                    t_ = ppre.tile([P, BC], f32, tag=f"h1_{m}", name="h1")
                    for k in (0, 1):
                        nc.tensor.matmul(t_[:], w1_sb[:, k, m, :], s1[k][:],
                                         start=(k == 0), stop=(k == 1))
                    h1.append(t_)
                for m in (0, 1):
                    s, _ = swishT(
                        f"b{m}", h1[m],
                        b1s[:, m:m + 1] if b1s is not None else None,
                        b1u[:, m:m + 1] if b1u is not None else None, pre)
                    s2.append(s)
                for m in (0, 1):
                    t_ = ppre.tile([P, BC], f32, tag=f"h2_{m}", name="h2")
                    for k in (0, 1):
                        nc.tensor.matmul(t_[:], w2_sb[:, k, m, :], s2[k][:],
                                         start=(k == 0), stop=(k == 1))
                    h2.append(t_)
                for m in (0, 1):
                    rt = pre.tile([P, BC], f32, tag=f"r_{m}_{branch}", name="rt")
                    nc.vector.tensor_add(rt[:], hb0[m][:], h2[m][:])
                    if b2u is not None:
                        nc.vector.tensor_scalar_add(rt[:], rt[:],
                                                    b2u[:, m:m + 1])
                    sig = pre.tile([P, BC], f32, tag=f"sig3_{m}", name="sig3")
                    nc.scalar.activation(sig[:], rt[:], AF.Sigmoid, bias=0.0,
                                         scale=BETA)
                    s = pre.tile([P, BC], f32, tag=f"s3_{m}_{branch}", name="s3")
                    nc.vector.tensor_mul(s[:], rt[:], sig[:])
                    s3.append(s)
                return s3

            with (
                tc.tile_pool(name="pre", bufs=2) as pre,
                tc.tile_pool(name="ppre", bufs=1, space="PSUM") as ppre,
                tc.tile_pool(name="ptab", bufs=1, space="PSUM") as ptab,
            ):
                s3k = resmlp_T(wkf_sb, bkfs_sb, bkfu_sb, kw1_sb, kb1s_sb,
                               kb1u_sb, kw2_sb, kb2u_sb, pre, ppre, "k")
                s3v = resmlp_T(wvf_sb, None, None, vw1_sb, None, None,
                               vw2_sb, None, pre, ppre, "v")
                # kqT[g, b] = sum_h s3k[h, b] * woq[h, g] + bq[g]
                pkq = ptab.tile([P, 2, BC], f32, tag="pkq")
                for c in (0, 1):
                    nc.tensor.matmul(pkq[:, c, :], woqk_sb[:, 0, c, :],
                                     s3k[0][:], start=True, stop=False)
                    nc.tensor.matmul(pkq[:, c, :], woqk_sb[:, 1, c, :],
                                     s3k[1][:], start=False, stop=False)
                    nc.tensor.matmul(pkq[:, c, :],
                                     bq_sb[0:1, c * P:(c + 1) * P],
                                     ones1[:], start=False, stop=True)
                nc.vector.tensor_copy(kqT16[:], pkq[:])
                # v16[b, f'] (chunked) = sum_h s3v[h, b] * wov[h, f']
                pv = ptab.tile([BC, 2, P], f32, tag="pv")
                for c in (0, 1):
                    for k in (0, 1):
                        nc.tensor.matmul(pv[:, c, :], s3v[k][:],
                                         wovv_sb[:, k, c, :],
                                         start=(k == 0), stop=(k == 1))
                nc.vector.tensor_copy(v16[:], pv[:])

            with (
                tc.tile_pool(name="pdot", bufs=2, space="PSUM") as pd_pool,
                tc.tile_pool(name="pout", bufs=2, space="PSUM") as po_pool,
            ):
                S_tiles = [None] * nsup
                r_tiles = [None] * nsup

                def pass2(s):
                    # S[s] *= r (all molecules of super s closed by now)
                    St = S_tiles[s]
                    rt = r_tiles[min(s + 1, nsup - 1)]
                    nc.vector.tensor_scalar_mul(St[:], St[:], rt[:])
                    out16 = op.tile([P, 2, SUP], f16, tag="out16")
                    for c in (0, 1):
                        po = po_pool.tile([P, 2, HB], f32, tag="po")
                        for b in (0, 1):
                            nc.tensor.matmul(
                                po[:, b, :], v16[:, c, :],
                                St[:, b * HB:(b + 1) * HB],
                                start=True, stop=True)
                        dst = out16[:, c, :].rearrange("p (b j) -> p b j", b=2)
                        nc.scalar.activation(dst, po[:], AF.Copy)
                    nc.sync.dma_start(out=ov[:, s, :, :], in_=out16[:])

                x_tiles = [None] * nsup
                m_tiles = [None] * nsup

                def fetch(s):
                    cols = slice(s * SUP, (s + 1) * SUP)
                    x16 = xp.tile([P, 2, SUP], f16, tag="x16")
                    nc.sync.dma_start(out=x16[:], in_=xv[:, s, :, :])
                    mk = mp.tile([BC, SUP], f8, tag="mk")
                    nc.sync.dma_start(out=mk[:], in_=mk_h[:, cols])
                    x_tiles[s], m_tiles[s] = x16, mk

                for s0 in range(min(2, nsup)):
                    x_tiles[s0], m_tiles[s0] = early[s0]
                for s in range(nsup):
                    if s + 2 < nsup:
                        fetch(s + 2)
                    if s >= 3:
                        pass2(s - 3)
                    x16, mk = x_tiles[s], m_tiles[s]

                    pd = pd_pool.tile([BC, 2, HB], f32, tag="pd")
                    for c in (0, 1):
                        for b in (0, 1):
                            nc.tensor.matmul(
                                pd[:, b, :], kqT16[:, c, :],
                                x16[:, c, b * HB:(b + 1) * HB],
                                start=(c == 0), stop=(c == 1),
                                skip_group_check=True)
                    e16 = ep.tile([BC, SUP], f16, tag="e16")
                    nc.scalar.activation(
                        e16[:].rearrange("p (b j) -> p b j", b=2), pd[:],
                        AF.Exp, bias=0.0, scale=INV_SQRT_F)
                    St = sp_.tile([BC, SUP], f16, tag="St")
                    part = rp.tile([BC, 1], f32, tag="part")
                    nc.vector.scalar_tensor_tensor(
                        St[:], e16[:], 1.0, mk[:], ALU.mult, ALU.mult,
                        accum_out=part[:])
                    S_tiles[s] = St
                    nc.vector.tensor_add(anorm_run[:], anorm_run[:], part[:])
                    rt = rp.tile([BC, 1], f32, tag="rt")
                    nc.vector.tensor_scalar_add(rt[:], anorm_run[:], EPS)
                    nc.vector.reciprocal(rt[:], rt[:])
                    r_tiles[s] = rt
                pass2(nsup - 3)
                pass2(nsup - 2)
                pass2(nsup - 1)

    nc.compile()
    return nc


def _prep_host(x, E, batch_seg, Wq, Wkf, bkf, Wvf, kW1, kb1, kW2, kb2, kWo,
               kbo, vW1, vW2, vWo):
    f32 = np.float32
    import ml_dtypes
    f16 = ml_dtypes.bfloat16
    bs = np.asarray(batch_seg).astype(np.int64)
    x = np.asarray(x, dtype=f32)
    N = x.shape[0]
    core_bounds = np.searchsorted(bs, np.arange(NCORES + 1) * BC, side="left")
    NCmax = int(np.max(np.diff(core_bounds)))
    nsup = max(1, -(-NCmax // SUP))
    NCpad = nsup * SUP

    xts, mks, evs = [], [], []
    E32 = np.asarray(E, dtype=f32)
    for c in range(NCORES):
        n0, n1 = core_bounds[c], core_bounds[c + 1]
        nc_ = n1 - n0
        xt = np.zeros((2 * P, NCpad), dtype=f16)
        xt[:, :nc_] = x[n0:n1].T.astype(f16)
        # interleave: [c_chunk*128+p, s*SUP+j] -> [p, s*(2*SUP)+c_chunk*SUP+j]
        xt = np.ascontiguousarray(
            xt.reshape(2, P, nsup, SUP).transpose(1, 2, 0, 3).reshape(P, -1))
        f8 = ml_dtypes.float8_e4m3fn
        mk = np.zeros((BC, NCpad), dtype=f8)
        mk[:, :nc_] = (bs[n0:n1][None, :]
                       == (np.arange(BC) + c * BC)[:, None]).astype(f8)
        xts.append(xt)
        mks.append(mk)
        evs.append(np.ascontiguousarray(E32[c * BC:(c + 1) * BC].reshape(1, BC)))

    def pack_w(W):
        A = np.asarray(W, dtype=f32)
        return np.ascontiguousarray(A.reshape(2, P, 2, P).transpose(3, 2, 0, 1))

    def pack_hw(M):
        # M [F(h), F(g)] -> [P(h'), k(h-half), c(g-half), P(g')]
        return np.ascontiguousarray(
            M.reshape(2, P, 2, P).transpose(1, 0, 2, 3))

    def pack_b(v, scale):
        a = (np.asarray(v, dtype=f32) * f32(scale)).astype(f32)
        return np.ascontiguousarray(a.reshape(2, P).T)

    Wq_, kWo_, vWo_ = (np.asarray(a, dtype=f32) for a in (Wq, kWo, vWo))
    woq = (kWo_.T @ Wq_).astype(f32)   # [h, g]
    wov = vWo_.T.astype(f32)           # [h, f]
    weights = dict(
        wkf=np.ascontiguousarray(np.asarray(Wkf, dtype=f32).reshape(F)[None, :]),
        wvf=np.ascontiguousarray(np.asarray(Wvf, dtype=f32).reshape(F)[None, :]),
        kw1=pack_w(kW1), kw2=pack_w(kW2),
        vw1=pack_w(vW1), vw2=pack_w(vW2),
        woqk=pack_hw(woq), wovv=pack_hw(wov),
        bq=np.ascontiguousarray(
            (np.asarray(kbo, dtype=f32) @ Wq_).reshape(1, F)),
        bkfs=pack_b(bkf, BETA), bkfu=pack_b(bkf, 1.0),
        kb1s=pack_b(kb1, BETA), kb1u=pack_b(kb1, 1.0),
        kb2u=pack_b(kb2, 1.0),
    )
    return nsup, xts, mks, evs, weights, core_bounds


_CACHE = {}
LAST_RESULT = None


def kernel(x, E, num_batch, batch_seg, Wq, Wkf, bkf, Wvf, kW1, kb1, kW2, kb2,
           kWo, kbo, vW1, vW2, vWo, **_ignored):
    from concourse.bass_utils import run_bass_kernel_spmd

    nsup, xts, mks, evs, weights, core_bounds = _prep_host(
        x, E, batch_seg, Wq, Wkf, bkf, Wvf, kW1, kb1, kW2, kb2, kWo, kbo,
        vW1, vW2, vWo)

    if nsup not in _CACHE:
        _CACHE[nsup] = _build_program(nsup)
    nc = _CACHE[nsup]

    in_maps = [
        dict(weights, x=xts[c], mk=mks[c], ev=evs[c])
        for c in range(NCORES)
    ]
    res = run_bass_kernel_spmd(nc, in_maps, core_ids=list(range(NCORES)))
    global LAST_RESULT
    LAST_RESULT = res

    NCpad = nsup * SUP
    N = np.asarray(x).shape[0]
    out = np.empty((N, F), dtype=np.float32)
    for c in range(NCORES):
        n0, n1 = core_bounds[c], core_bounds[c + 1]
        o = np.asarray(res.results[c]["out"])
        # [p, s*(2*SUP)+cc*SUP+j] -> [cc*128+p, s*SUP+j]
        oT = o.reshape(P, nsup, 2, SUP).transpose(2, 0, 1, 3).reshape(F, NCpad)
        out[n0:n1] = oT[:, :n1 - n0].T.astype(np.float32)
    return out
